# revision 1
# baseline (speedup 1.0000x reference)
"""Trainium2 Bass kernel for a 2-layer spiking LSTM (SLSTM) + FC readout.

Contract: kernel(**inputs) takes the FULL unsharded inputs and returns
the FULL [256, 8] output. Internally the batch is sharded 32-per-core
across 8 NeuronCores (data parallel, weights replicated); the T=400
time scan runs fully SBUF-resident per core.

Layout notes (per core, BL=32):
  - Gates for one layer live in one PSUM bank [128, 512] f32:
      partition p = 32*jc + b   (jc = output h-chunk of 128, b = batch)
      free      n = 128*gt + hp (gt in order [i, f, o, g], hp = h%128)
    produced by 4-way PE column tiling: col group jc computes h-chunk jc
    with the state slice [128, 32] stationary and reordered weights
    streaming (N=512).
  - Matmul operands are bf16 (weights cast on host; states cast during
    the PSUM->SBUF move after the PE transpose); accumulation is f32.
  - States (syn/mem) stay f32 in SBUF [128, 128] = [(jc, b), hp].
  - Biases: layer 1 b_ih+b_hh ride along as an extra ones-column of x
    (K=15); layer 2 bias is a K=1 ones matmul that opens the PSUM
    accumulation group.
  - Emission order pipelines steps: G2's bias+mem2 rounds are issued
    right after G1 so the PE streams them while layer-1 elementwise
    runs; the spk1-dependent rounds go last.
"""

import sys

sys.path.insert(0, "/opt/trn_rl_repo")

import numpy as np
import ml_dtypes

T, B, I, H, C = 400, 256, 14, 512, 8
N_CORES = 8
BL = B // N_CORES  # 32
GATE_PERM = [0, 1, 3, 2]  # PyTorch gate rows [i,f,g,o] -> our order [i,f,o,g]

_cache = {}


def _reorder_w(W: np.ndarray) -> np.ndarray:
    """[2048, Kin] (rows i,f,g,o) -> [128, KC*2048] bf16 streaming layout.

    free index = kc*2048 + jc*512 + gt*128 + hp, partition = k (h within
    contraction chunk kc)."""
    Kin = W.shape[1]
    KC = Kin // 128
    Wg = W.reshape(4, 4, 128, KC, 128)[GATE_PERM]  # [gt, jc, hp, kc, k]
    return np.ascontiguousarray(
        Wg.transpose(4, 3, 1, 0, 2).reshape(128, KC * 2048)
    ).astype(ml_dtypes.bfloat16)


def _reorder_w_small(Waug: np.ndarray) -> np.ndarray:
    """[2048, Kin<=128] -> [Kin, 2048] bf16; free = jc*512 + gt*128 + hp."""
    Kin = Waug.shape[1]
    Wg = Waug.reshape(4, 4, 128, Kin)[GATE_PERM]  # [gt, jc, hp, k]
    return np.ascontiguousarray(
        Wg.transpose(3, 1, 0, 2).reshape(Kin, 2048)
    ).astype(ml_dtypes.bfloat16)


def _reorder_b(b: np.ndarray) -> np.ndarray:
    bg = b.reshape(4, 4, 128)[GATE_PERM]  # [gt, jc, hp]
    return np.ascontiguousarray(
        bg.transpose(1, 0, 2).reshape(1, 2048)
    ).astype(ml_dtypes.bfloat16)


def build_nc(thr1: float, thr2: float, t_steps: int):
    import concourse.bacc as bacc
    import concourse.mybir as mybir
    from concourse import tile, masks
    from concourse.tile import add_dep_helper

    f32 = mybir.dt.float32
    bf16 = mybir.dt.bfloat16
    AF = mybir.ActivationFunctionType
    OP = mybir.AluOpType

    nc = bacc.Bacc("TRN2", target_bir_lowering=False, debug=False,
                   num_devices=N_CORES)

    d_x = nc.dram_tensor("xin", [15, t_steps * BL], bf16, kind="ExternalInput")
    d_wih1 = nc.dram_tensor("wih1", [15, 2048], bf16, kind="ExternalInput")
    d_whh1 = nc.dram_tensor("whh1", [128, 4 * 2048], bf16,
                            kind="ExternalInput")
    d_w2 = nc.dram_tensor("w2", [128, 8 * 2048], bf16, kind="ExternalInput")
    d_b2 = nc.dram_tensor("b2r", [1, 2048], bf16, kind="ExternalInput")
    d_out = nc.dram_tensor("msum", [128, 128], f32, kind="ExternalOutput")

    with tile.TileContext(nc) as tc:
        with (
            tc.tile_pool(name="const", bufs=1) as cpool,
            tc.tile_pool(name="state", bufs=1) as spool,
            tc.tile_pool(name="gs", bufs=2) as gspool,
            tc.tile_pool(name="tmp", bufs=2) as tpool,
            tc.tile_pool(name="tsb", bufs=2) as tsbpool,
            tc.tile_pool(name="g1", bufs=2, space="PSUM") as g1pool,
            tc.tile_pool(name="g2", bufs=2, space="PSUM") as g2pool,
            tc.tile_pool(name="tp", bufs=2, space="PSUM") as tppool,
        ):
            x_sb = cpool.tile([15, t_steps * BL], bf16, tag="x")
            wih1 = cpool.tile([15, 2048], bf16, tag="wih1")
            whh1 = cpool.tile([128, 4 * 2048], bf16, tag="whh1")
            w2 = cpool.tile([128, 8 * 2048], bf16, tag="w2")
            b2r = cpool.tile([1, 2048], bf16, tag="b2r")
            ident = cpool.tile([128, 128], f32, tag="ident")
            ones = cpool.tile([1, BL], bf16, tag="ones")

            nc.sync.dma_start(x_sb[:], d_x[:])
            nc.sync.dma_start(wih1[:], d_wih1[:])
            nc.sync.dma_start(whh1[:], d_whh1[:])
            nc.sync.dma_start(w2[:], d_w2[:])
            nc.sync.dma_start(b2r[:], d_b2[:])
            masks.make_identity(nc, ident[:])
            nc.gpsimd.memset(ones[:], 1.0)

            syn1 = spool.tile([128, 128], f32, tag="syn1")
            mem1 = spool.tile([128, 128], f32, tag="mem1")
            syn2 = spool.tile([128, 128], f32, tag="syn2")
            mem2 = spool.tile([128, 128], f32, tag="mem2")
            msum = spool.tile([128, 128], f32, tag="msum")
            m1T = spool.tile([128, 128], bf16, tag="m1T0")
            m2T = spool.tile([128, 128], bf16, tag="m2T0")
            for s in (syn1, mem1, syn2, mem2, msum, m1T, m2T):
                nc.vector.memset(s[:], 0.0)

            def mm(psum, lhs, rhs, jc, start, stop):
                return nc.tensor.matmul(
                    psum[32 * jc:32 * jc + 32, :], lhs, rhs,
                    start=start, stop=stop, tile_position=(0, 32 * jc),
                    skip_group_check=True)

            def x_round(g1t, t):
                xsl = x_sb[:, t * BL:(t + 1) * BL]
                return [mm(g1t, xsl, wih1[:, 512 * jc:512 * jc + 512], jc,
                           True, False) for jc in range(4)]

            def bias_round(g2t):
                return [mm(g2t, ones[0:1, :],
                           b2r[0:1, 512 * jc:512 * jc + 512], jc,
                           True, False) for jc in range(4)]

            # open step-0 accumulation groups (x part + layer-2 bias)
            g1 = g1pool.tile([128, 512], f32, tag="g1")
            # HAM warm-up: ~4us of junk matmuls so the PE clock ungates
            # before the scan starts (the x round below overwrites g1)
            for _ in range(4):
                bias_round(g1)
            x_round(g1, 0)
            g2 = g2pool.tile([128, 512], f32, tag="g2")
            bias_round(g2)

            def junk_mm(target, anchor):
                """Tiny matmul into `target` (later overwritten) that fires
                once `anchor` completes — keeps the PE activity monitor from
                re-throttling the clock during elementwise chains."""
                j = nc.tensor.matmul(
                    target[0:32, 0:32], ones[0:1, :], b2r[0:1, 0:32],
                    start=True, stop=True, tile_position=(0, 0),
                    skip_group_check=True)
                add_dep_helper(anchor.ins, j.ins, sync=True,
                               reason="ham keepalive")
                return j

            ew2_anchors = []
            for t in range(t_steps):
                # ---- G1 tail: W_hh1 rounds (x round already emitted) ----
                for kc in range(4):
                    lhs = m1T[:, 32 * kc:32 * kc + 32]
                    for jc in range(4):
                        off = 2048 * kc + 512 * jc
                        mm(g1, lhs, whh1[:, off:off + 512], jc,
                           False, kc == 3)

                # keepalive matmuls paced by the previous step's layer-2
                # elementwise chain (fills the PE while ew2[t-1] finishes)
                tp2 = tppool.tile([128, 128], f32, tag="tp2")
                for anc in ew2_anchors:
                    junk_mm(tp2, anc)

                # ---- deferred transpose of last step's mem2 ----
                # (emitted here so in the PE FIFO it sits after G1[t] but
                # before the G2 rounds that consume m2T)
                if t > 0:
                    nc.tensor.transpose(tp2[:], mem2[:], ident[:])
                    m2T_new = tsbpool.tile([128, 128], bf16, tag="m2T")
                    nc.vector.tensor_copy(m2T_new[:], tp2[:])
                    m2T = m2T_new

                # ---- G2: W_hh2 @ mem2 rounds (bias already emitted) ----
                # (the PE streams these during layer-1 ew)
                for kc in range(4):  # w2 chunks 4..7 = W_hh2
                    lhs = m2T[:, 32 * kc:32 * kc + 32]
                    for jc in range(4):
                        off = 2048 * (4 + kc) + 512 * jc
                        mm(g2, lhs, w2[:, off:off + 512], jc, False, False)

                # ---- layer 1 elementwise ----
                gs1 = gspool.tile([128, 512], f32, tag="gs1")
                r1 = tpool.tile([128, 128], f32, tag="r1")
                nc.vector.tensor_scalar(r1[:], mem1[:], thr1, thr1,
                                        OP.is_gt, OP.mult)
                i_sg = nc.scalar.activation(gs1[:, 0:384], g1[:, 0:384],
                                            AF.Sigmoid)
                i_tg = nc.scalar.activation(gs1[:, 384:512], g1[:, 384:512],
                                            AF.Tanh)
                si, sf = gs1[:, 0:128], gs1[:, 128:256]
                so, tg = gs1[:, 256:384], gs1[:, 384:512]

                p2 = tpool.tile([128, 128], f32, tag="p2")
                nc.gpsimd.tensor_mul(p2[:], sf, syn1[:])
                p1 = tpool.tile([128, 128], f32, tag="p1")
                i_p1 = nc.vector.tensor_mul(p1[:], si, tg)
                i_sy = nc.vector.tensor_add(syn1[:], p1[:], p2[:])
                tc1 = tpool.tile([128, 128], f32, tag="tc1")
                i_tc = nc.scalar.activation(tc1[:], syn1[:], AF.Tanh)
                ht1 = tpool.tile([128, 128], f32, tag="ht1")
                i_ht = nc.vector.tensor_mul(ht1[:], so, tc1[:])
                nc.vector.tensor_sub(mem1[:], ht1[:], r1[:])

                # open next step's groups now — these two rounds have no
                # dependencies and fill the PE while the ew1 tail finishes
                if t + 1 < t_steps:
                    g1_next = g1pool.tile([128, 512], f32, tag="g1")
                    x_round(g1_next, t + 1)
                    g2_next = g2pool.tile([128, 512], f32, tag="g2")
                    bias_round(g2_next)

                # keepalive matmuls paced by the layer-1 elementwise chain
                tp1 = tppool.tile([128, 128], f32, tag="tp1")
                for anc in (i_sg, i_sy, i_tc, i_ht):
                    junk_mm(tp1, anc)

                # transpose mem1; spike threshold + bf16 casts fused into
                # the PSUM->SBUF moves
                nc.tensor.transpose(tp1[:], mem1[:], ident[:])
                spk1T = tsbpool.tile([128, 128], bf16, tag="spk")
                nc.vector.tensor_scalar(spk1T[:], tp1[:], thr1, None, OP.is_gt)
                m1T_new = tsbpool.tile([128, 128], bf16, tag="m1T")
                nc.scalar.copy(m1T_new[:], tp1[:])

                # ---- G2 tail: W_ih2 @ spk1 rounds (close group) ----
                for kc in range(4):  # w2 chunks 0..3 = W_ih2
                    lhs = spk1T[:, 32 * kc:32 * kc + 32]
                    for jc in range(4):
                        off = 2048 * kc + 512 * jc
                        mm(g2, lhs, w2[:, off:off + 512], jc, False, kc == 3)

                # ---- layer 2 elementwise ----
                gs2 = gspool.tile([128, 512], f32, tag="gs2")
                r2 = tpool.tile([128, 128], f32, tag="r2")
                nc.vector.tensor_scalar(r2[:], mem2[:], thr2, thr2,
                                        OP.is_gt, OP.mult)
                i_sg2 = nc.scalar.activation(gs2[:, 0:384], g2[:, 0:384],
                                             AF.Sigmoid)
                nc.scalar.activation(gs2[:, 384:512], g2[:, 384:512], AF.Tanh)
                si2, sf2 = gs2[:, 0:128], gs2[:, 128:256]
                so2, tg2 = gs2[:, 256:384], gs2[:, 384:512]

                q2 = tpool.tile([128, 128], f32, tag="q2")
                nc.gpsimd.tensor_mul(q2[:], sf2, syn2[:])
                q1 = tpool.tile([128, 128], f32, tag="q1")
                nc.vector.tensor_mul(q1[:], si2, tg2)
                i_sy2 = nc.vector.tensor_add(syn2[:], q1[:], q2[:])
                tc2 = tpool.tile([128, 128], f32, tag="tc2")
                i_tc2 = nc.scalar.activation(tc2[:], syn2[:], AF.Tanh)
                ht2 = tpool.tile([128, 128], f32, tag="ht2")
                nc.vector.tensor_mul(ht2[:], so2, tc2[:])
                nc.vector.tensor_sub(mem2[:], ht2[:], r2[:])

                nc.gpsimd.tensor_add(msum[:], msum[:], mem2[:])

                m1T = m1T_new
                ew2_anchors = [i_sg2, i_sy2, i_tc2]
                if t + 1 < t_steps:
                    g1, g2 = g1_next, g2_next

            nc.sync.dma_start(d_out[:], msum[:])

    nc.compile()
    return nc


def prep_core_inputs(x, W_ih1, W_hh1, b_ih1, b_hh1, W_ih2, W_hh2,
                     b_ih2, b_hh2, t_steps):
    """Shared (weight) arrays + per-core x shards."""
    b1 = np.asarray(b_ih1, np.float32) + np.asarray(b_hh1, np.float32)
    wih1_aug = np.concatenate(
        [np.asarray(W_ih1, np.float32), b1[:, None]], axis=1)  # [2048, 15]
    wih1_r = _reorder_w_small(wih1_aug)  # [15, 2048]
    whh1_r = _reorder_w(np.asarray(W_hh1, np.float32))  # [128, 8192]
    w2cat = np.concatenate(
        [np.asarray(W_ih2, np.float32), np.asarray(W_hh2, np.float32)],
        axis=1)  # [2048, 1024]
    w2_r = _reorder_w(w2cat)  # [128, 16384]
    b2_r = _reorder_b(np.asarray(b_ih2, np.float32)
                      + np.asarray(b_hh2, np.float32))  # [1, 2048]

    x = np.asarray(x, np.float32)[:t_steps]
    in_maps = []
    for c in range(N_CORES):
        xs = x[:, c * BL:(c + 1) * BL, :]  # [T, 32, 14]
        xt = np.ascontiguousarray(
            xs.transpose(2, 0, 1).reshape(I, t_steps * BL))
        x_aug = np.concatenate(
            [xt, np.ones((1, t_steps * BL), np.float32)],
            axis=0).astype(ml_dtypes.bfloat16)  # [15, T*32]
        in_maps.append({
            "xin": x_aug,
            "wih1": wih1_r,
            "whh1": whh1_r,
            "w2": w2_r,
            "b2r": b2_r,
        })
    return in_maps


def unpack_msum(msum: np.ndarray, t_steps: int) -> np.ndarray:
    """[128, 128] device accumulator -> [32, 512] mem2 mean."""
    return (msum.reshape(4, 32, 128).transpose(1, 0, 2).reshape(32, 512)
            / np.float32(t_steps))


def kernel(x, W_ih1, W_hh1, b_ih1, b_hh1, thr1,
           W_ih2, W_hh2, b_ih2, b_hh2, thr2, W_fc, b_fc):
    from concourse.bass_utils import run_bass_kernel_spmd

    t_steps = x.shape[0]
    key = (float(thr1), float(thr2), t_steps)
    if key not in _cache:
        _cache[key] = build_nc(float(thr1), float(thr2), t_steps)
    nc = _cache[key]

    in_maps = prep_core_inputs(x, W_ih1, W_hh1, b_ih1, b_hh1,
                               W_ih2, W_hh2, b_ih2, b_hh2, t_steps)
    res = run_bass_kernel_spmd(nc, in_maps, list(range(N_CORES)))

    W_fc = np.asarray(W_fc, np.float32)
    b_fc = np.asarray(b_fc, np.float32)
    out = np.empty((B, C), np.float32)
    for c in range(N_CORES):
        mean_c = unpack_msum(res.results[c]["msum"], t_steps)  # [32, 512]
        out[c * BL:(c + 1) * BL] = mean_c @ W_fc.T + b_fc
    return out



# revision 3
# speedup vs baseline: 1.0095x; 1.0095x over previous
"""Trainium2 Bass kernel for a 2-layer spiking LSTM (SLSTM) + FC readout.

Contract: kernel(**inputs) takes the FULL unsharded inputs and returns
the FULL [256, 8] output. Internally the batch is sharded 32-per-core
across 8 NeuronCores (data parallel, weights replicated); the T=400
time scan runs fully SBUF-resident per core.

Layout notes (per core, BL=32):
  - Gates for one layer live in one PSUM bank [128, 512] f32:
      partition p = 32*jc + b   (jc = output h-chunk of 128, b = batch)
      free      n = 128*gt + hp (gt in order [i, f, 2g, o], hp = h%128)
    produced by 4-way PE column tiling: col group jc computes h-chunk jc
    with the state slice [128, 32] stationary and reordered weights
    streaming (N=512).
  - The g-gate's weight/bias rows are pre-scaled by 2 on the host so a
    single sigmoid over [i, f, 2g] gives tanh(g) = 2*sigmoid(2g) - 1
    via one cheap DVE tensor_scalar — one fewer ACT op on the critical
    chain per layer.
  - Elementwise state (syn, gate outputs, h) is bf16 so DVE runs at
    2x; PE transposes h (not mem), and the reset subtraction happens in
    the transposed domain: mT = hT - rT, spkT = hT > (thr + rT), with
    rT / thr+rT precomputed off the critical chain from mT[t-1].
  - Biases: layer 1 b_ih+b_hh ride along as an extra ones-column of x
    (K=15); layer 2 bias is a K=1 ones matmul that opens the PSUM
    accumulation group.
"""

import sys

sys.path.insert(0, "/opt/trn_rl_repo")

import numpy as np
import ml_dtypes

T, B, I, H, C = 400, 256, 14, 512, 8
N_CORES = 8
BL = B // N_CORES  # 32
GATE_PERM = [0, 1, 2, 3]  # PyTorch gate rows [i,f,g,o] kept in order
GATE_SCALE = np.array([1.0, 1.0, 2.0, 1.0], np.float32)  # 2x on g rows

_cache = {}


def _scale_g(W: np.ndarray) -> np.ndarray:
    """Scale the g-gate rows ([2H:3H]) of a [4H, K] weight by 2."""
    W = np.asarray(W, np.float32).copy()
    W[2 * H:3 * H] *= 2.0
    return W


def _reorder_w(W: np.ndarray) -> np.ndarray:
    """[2048, Kin] (rows i,f,g,o) -> [128, KC*2048] bf16 streaming layout.

    free index = kc*2048 + jc*512 + gt*128 + hp, partition = k (h within
    contraction chunk kc)."""
    Kin = W.shape[1]
    KC = Kin // 128
    Wg = W.reshape(4, 4, 128, KC, 128)[GATE_PERM]  # [gt, jc, hp, kc, k]
    return np.ascontiguousarray(
        Wg.transpose(4, 3, 1, 0, 2).reshape(128, KC * 2048)
    ).astype(ml_dtypes.bfloat16)


def _reorder_w_small(Waug: np.ndarray) -> np.ndarray:
    """[2048, Kin<=128] -> [Kin, 2048] bf16; free = jc*512 + gt*128 + hp."""
    Kin = Waug.shape[1]
    Wg = Waug.reshape(4, 4, 128, Kin)[GATE_PERM]  # [gt, jc, hp, k]
    return np.ascontiguousarray(
        Wg.transpose(3, 1, 0, 2).reshape(Kin, 2048)
    ).astype(ml_dtypes.bfloat16)


def _reorder_b(b: np.ndarray) -> np.ndarray:
    bg = b.reshape(4, 4, 128)[GATE_PERM]  # [gt, jc, hp]
    return np.ascontiguousarray(
        bg.transpose(1, 0, 2).reshape(1, 2048)
    ).astype(ml_dtypes.bfloat16)


def build_nc(thr1: float, thr2: float, t_steps: int):
    import concourse.bacc as bacc
    import concourse.mybir as mybir
    from concourse import tile, masks
    from concourse.tile import add_dep_helper

    f32 = mybir.dt.float32
    bf16 = mybir.dt.bfloat16
    AF = mybir.ActivationFunctionType
    OP = mybir.AluOpType

    nc = bacc.Bacc("TRN2", target_bir_lowering=False, debug=False,
                   num_devices=N_CORES)

    d_x = nc.dram_tensor("xin", [15, t_steps * BL], bf16, kind="ExternalInput")
    d_wih1 = nc.dram_tensor("wih1", [15, 2048], bf16, kind="ExternalInput")
    d_whh1 = nc.dram_tensor("whh1", [128, 4 * 2048], bf16,
                            kind="ExternalInput")
    d_w2 = nc.dram_tensor("w2", [128, 8 * 2048], bf16, kind="ExternalInput")
    d_b2 = nc.dram_tensor("b2r", [1, 2048], bf16, kind="ExternalInput")
    d_out = nc.dram_tensor("msum", [128, 128], f32, kind="ExternalOutput")

    with tile.TileContext(nc) as tc:
        with (
            tc.tile_pool(name="const", bufs=1) as cpool,
            tc.tile_pool(name="state", bufs=1) as spool,
            tc.tile_pool(name="gs", bufs=2) as gspool,
            tc.tile_pool(name="tmp", bufs=2) as tpool,
            tc.tile_pool(name="tsb", bufs=2) as tsbpool,
            tc.tile_pool(name="g1", bufs=2, space="PSUM") as g1pool,
            tc.tile_pool(name="g2", bufs=2, space="PSUM") as g2pool,
            tc.tile_pool(name="tp", bufs=2, space="PSUM") as tppool,
            tc.tile_pool(name="jp", bufs=2, space="PSUM") as jpool,
        ):
            x_sb = cpool.tile([15, t_steps * BL], bf16, tag="x")
            wih1 = cpool.tile([15, 2048], bf16, tag="wih1")
            whh1 = cpool.tile([128, 4 * 2048], bf16, tag="whh1")
            w2 = cpool.tile([128, 8 * 2048], bf16, tag="w2")
            b2r = cpool.tile([1, 2048], bf16, tag="b2r")
            ident = cpool.tile([128, 128], bf16, tag="ident")
            ones = cpool.tile([1, BL], bf16, tag="ones")

            nc.sync.dma_start(x_sb[:], d_x[:])
            nc.sync.dma_start(wih1[:], d_wih1[:])
            nc.sync.dma_start(whh1[:], d_whh1[:])
            nc.sync.dma_start(w2[:], d_w2[:])
            nc.sync.dma_start(b2r[:], d_b2[:])
            masks.make_identity(nc, ident[:])
            nc.gpsimd.memset(ones[:], 1.0)

            # states: syn in bf16 (DVE 2x); transposed mem in bf16
            syn1 = spool.tile([128, 128], bf16, tag="syn1")
            syn2 = spool.tile([128, 128], bf16, tag="syn2")
            m1T = spool.tile([128, 128], bf16, tag="m1T0")
            m2T = spool.tile([128, 128], bf16, tag="m2T0")
            r1T = spool.tile([128, 128], bf16, tag="r1T")
            tp1T = spool.tile([128, 128], bf16, tag="tp1T")  # thr1 + r1T
            r2T = spool.tile([128, 128], bf16, tag="r2T")
            mem2 = spool.tile([128, 128], f32, tag="mem2")
            r2 = spool.tile([128, 128], f32, tag="r2")
            msum = spool.tile([128, 128], f32, tag="msum")
            for s in (syn1, syn2, m1T, m2T, r1T, r2T):
                nc.vector.memset(s[:], 0.0)
            nc.vector.memset(tp1T[:], thr1)
            for s in (mem2, r2, msum):
                nc.vector.memset(s[:], 0.0)

            def mm(psum, lhs, rhs, jc, start, stop):
                return nc.tensor.matmul(
                    psum[32 * jc:32 * jc + 32, :], lhs, rhs,
                    start=start, stop=stop, tile_position=(0, 32 * jc),
                    skip_group_check=True)

            def x_round(g1t, t):
                xsl = x_sb[:, t * BL:(t + 1) * BL]
                return [mm(g1t, xsl, wih1[:, 512 * jc:512 * jc + 512], jc,
                           True, False) for jc in range(4)]

            def bias_round(g2t):
                return [mm(g2t, ones[0:1, :],
                           b2r[0:1, 512 * jc:512 * jc + 512], jc,
                           True, False) for jc in range(4)]

            # open step-0 accumulation groups (x part + layer-2 bias)
            g1 = g1pool.tile([128, 512], f32, tag="g1")
            # HAM warm-up: ~4us of junk matmuls so the PE clock ungates
            # before the scan starts (the x round below overwrites g1)
            for _ in range(4):
                bias_round(g1)
            x_round(g1, 0)
            g2 = g2pool.tile([128, 512], f32, tag="g2")
            bias_round(g2)

            def junk_mm(target, anchor):
                """Tiny matmul into `target` that fires once `anchor`
                completes — keeps the PE activity monitor from
                re-throttling the clock during elementwise chains."""
                j = nc.tensor.matmul(
                    target[0:32, 0:32], ones[0:1, :], b2r[0:1, 0:32],
                    start=True, stop=True, tile_position=(0, 0),
                    skip_group_check=True)
                add_dep_helper(anchor.ins, j.ins, sync=True,
                               reason="ham keepalive")
                return j

            for t in range(t_steps):
                # ---- G1 tail: W_hh1 rounds (x round already emitted) ----
                for kc in range(4):
                    lhs = m1T[:, 32 * kc:32 * kc + 32]
                    for jc in range(4):
                        off = 2048 * kc + 512 * jc
                        mm(g1, lhs, whh1[:, off:off + 512], jc,
                           False, kc == 3)

                # ---- deferred: transpose last step's ht2, derive m2T and
                # the transposed reset for this step's layer 2 ----
                if t > 0:
                    tph = tppool.tile([128, 128], bf16, tag="tp")
                    nc.tensor.transpose(tph[:], ht2_prev[:], ident[:])
                    m2T_new = tsbpool.tile([128, 128], bf16, tag="m2T")
                    nc.vector.tensor_tensor(m2T_new[:], tph[:], r2T[:],
                                            OP.subtract)
                    m2T = m2T_new
                    nc.vector.tensor_scalar(r2T[:], m2T[:], thr2, thr2,
                                            OP.is_gt, OP.mult)

                # ---- G2: W_hh2 @ mem2 rounds (bias already emitted) ----
                for kc in range(4):  # w2 chunks 4..7 = W_hh2
                    lhs = m2T[:, 32 * kc:32 * kc + 32]
                    for jc in range(4):
                        off = 2048 * (4 + kc) + 512 * jc
                        mm(g2, lhs, w2[:, off:off + 512], jc, False, False)

                # ---- layer 1 elementwise ----
                # gates order [i, f, 2g, o]: one sigmoid over [0:384]
                gs1 = gspool.tile([128, 512], bf16, tag="gs1")
                i_sg = nc.scalar.activation(gs1[:, 0:384], g1[:, 0:384],
                                            AF.Sigmoid)
                nc.scalar.activation(gs1[:, 384:512], g1[:, 384:512],
                                     AF.Sigmoid)
                si, sf = gs1[:, 0:128], gs1[:, 128:256]
                sg, so = gs1[:, 256:384], gs1[:, 384:512]

                tg = tpool.tile([128, 128], bf16, tag="tg")
                nc.vector.tensor_scalar(tg[:], sg, 2.0, 1.0,
                                        OP.mult, OP.subtract)
                p2 = tpool.tile([128, 128], bf16, tag="p2")
                nc.gpsimd.tensor_mul(p2[:], sf, syn1[:])
                p1 = tpool.tile([128, 128], bf16, tag="p1")
                nc.vector.tensor_mul(p1[:], si, tg[:])
                i_sy = nc.vector.tensor_add(syn1[:], p1[:], p2[:])
                tc1 = tpool.tile([128, 128], bf16, tag="tc1")
                i_tc = nc.scalar.activation(tc1[:], syn1[:], AF.Tanh)
                ht1 = tpool.tile([128, 128], bf16, tag="ht1")
                nc.vector.tensor_mul(ht1[:], so, tc1[:])

                # open next step's groups now — these two rounds have no
                # dependencies and fill the PE while the ew1 tail finishes
                if t + 1 < t_steps:
                    g1_next = g1pool.tile([128, 512], f32, tag="g1")
                    x_round(g1_next, t + 1)
                    g2_next = g2pool.tile([128, 512], f32, tag="g2")
                    bias_round(g2_next)

                # keepalive matmuls paced by the layer-1 ACT chain
                jt = jpool.tile([32, 32], f32, tag="jt")
                for anc in (i_sg, i_sy, i_tc):
                    junk_mm(jt, anc)

                # transpose h (not mem): spike test and reset-subtract
                # both happen in the transposed domain
                tpp = tppool.tile([128, 128], bf16, tag="tp")
                nc.tensor.transpose(tpp[:], ht1[:], ident[:])
                spk1T = tsbpool.tile([128, 128], bf16, tag="spk")
                nc.vector.tensor_tensor(spk1T[:], tpp[:], tp1T[:], OP.is_gt)
                m1T_new = tsbpool.tile([128, 128], bf16, tag="m1T")
                nc.vector.tensor_tensor(m1T_new[:], tpp[:], r1T[:],
                                        OP.subtract)
                # off-chain: next step's transposed reset for layer 1
                nc.vector.tensor_scalar(r1T[:], m1T_new[:], thr1, thr1,
                                        OP.is_gt, OP.mult)
                nc.vector.tensor_scalar(tp1T[:], r1T[:], thr1, None, OP.add)

                # ---- G2 tail: W_ih2 @ spk1 rounds (close group) ----
                for kc in range(4):  # w2 chunks 0..3 = W_ih2
                    lhs = spk1T[:, 32 * kc:32 * kc + 32]
                    for jc in range(4):
                        off = 2048 * kc + 512 * jc
                        mm(g2, lhs, w2[:, off:off + 512], jc, False, kc == 3)

                # ---- layer 2 elementwise ----
                gs2 = gspool.tile([128, 512], bf16, tag="gs2")
                i_sg2 = nc.scalar.activation(gs2[:, 0:384], g2[:, 0:384],
                                             AF.Sigmoid)
                nc.scalar.activation(gs2[:, 384:512], g2[:, 384:512],
                                     AF.Sigmoid)
                si2, sf2 = gs2[:, 0:128], gs2[:, 128:256]
                sg2, so2 = gs2[:, 256:384], gs2[:, 384:512]

                tg2 = tpool.tile([128, 128], bf16, tag="tg2")
                nc.vector.tensor_scalar(tg2[:], sg2, 2.0, 1.0,
                                        OP.mult, OP.subtract)
                q2 = tpool.tile([128, 128], bf16, tag="q2")
                nc.gpsimd.tensor_mul(q2[:], sf2, syn2[:])
                q1 = tpool.tile([128, 128], bf16, tag="q1")
                nc.vector.tensor_mul(q1[:], si2, tg2[:])
                i_sy2 = nc.vector.tensor_add(syn2[:], q1[:], q2[:])
                tc2 = tpool.tile([128, 128], bf16, tag="tc2")
                i_tc2 = nc.scalar.activation(tc2[:], syn2[:], AF.Tanh)
                ht2 = tpool.tile([128, 128], bf16, tag="ht2")
                nc.vector.tensor_mul(ht2[:], so2, tc2[:])

                # keepalive matmuls paced by the layer-2 ACT chain
                jt2 = jpool.tile([32, 32], f32, tag="jt")
                for anc in (i_sg2, i_sy2, i_tc2):
                    junk_mm(jt2, anc)

                # off-chain: untransposed mem2 (f32) for the running sum,
                # and the untransposed reset for the next step
                nc.vector.tensor_tensor(mem2[:], ht2[:], r2[:], OP.subtract)
                nc.vector.tensor_scalar(r2[:], mem2[:], thr2, thr2,
                                        OP.is_gt, OP.mult)
                nc.gpsimd.tensor_add(msum[:], msum[:], mem2[:])

                ht2_prev = ht2
                if t + 1 < t_steps:
                    g1, g2 = g1_next, g2_next

            nc.sync.dma_start(d_out[:], msum[:])

    nc.compile()
    return nc


def prep_core_inputs(x, W_ih1, W_hh1, b_ih1, b_hh1, W_ih2, W_hh2,
                     b_ih2, b_hh2, t_steps):
    """Shared (weight) arrays + per-core x shards."""
    b1 = (np.asarray(b_ih1, np.float32) + np.asarray(b_hh1, np.float32))
    b1 = _scale_g(b1[:, None])[:, 0]
    wih1_aug = np.concatenate(
        [_scale_g(W_ih1), b1[:, None]], axis=1)  # [2048, 15]
    wih1_r = _reorder_w_small(wih1_aug)  # [15, 2048]
    whh1_r = _reorder_w(_scale_g(W_hh1))  # [128, 8192]
    w2cat = np.concatenate(
        [_scale_g(W_ih2), _scale_g(W_hh2)], axis=1)  # [2048, 1024]
    w2_r = _reorder_w(w2cat)  # [128, 16384]
    b2 = _scale_g((np.asarray(b_ih2, np.float32)
                   + np.asarray(b_hh2, np.float32))[:, None])[:, 0]
    b2_r = _reorder_b(b2)  # [1, 2048]

    x = np.asarray(x, np.float32)[:t_steps]
    in_maps = []
    for c in range(N_CORES):
        xs = x[:, c * BL:(c + 1) * BL, :]  # [T, 32, 14]
        xt = np.ascontiguousarray(
            xs.transpose(2, 0, 1).reshape(I, t_steps * BL))
        x_aug = np.concatenate(
            [xt, np.ones((1, t_steps * BL), np.float32)],
            axis=0).astype(ml_dtypes.bfloat16)  # [15, T*32]
        in_maps.append({
            "xin": x_aug,
            "wih1": wih1_r,
            "whh1": whh1_r,
            "w2": w2_r,
            "b2r": b2_r,
        })
    return in_maps


def unpack_msum(msum: np.ndarray, t_steps: int) -> np.ndarray:
    """[128, 128] device accumulator -> [32, 512] mem2 mean."""
    return (msum.reshape(4, 32, 128).transpose(1, 0, 2).reshape(32, 512)
            / np.float32(t_steps))


def kernel(x, W_ih1, W_hh1, b_ih1, b_hh1, thr1,
           W_ih2, W_hh2, b_ih2, b_hh2, thr2, W_fc, b_fc):
    from concourse.bass_utils import run_bass_kernel_spmd

    t_steps = x.shape[0]
    key = (float(thr1), float(thr2), t_steps)
    if key not in _cache:
        _cache[key] = build_nc(float(thr1), float(thr2), t_steps)
    nc = _cache[key]

    in_maps = prep_core_inputs(x, W_ih1, W_hh1, b_ih1, b_hh1,
                               W_ih2, W_hh2, b_ih2, b_hh2, t_steps)
    res = run_bass_kernel_spmd(nc, in_maps, list(range(N_CORES)))

    W_fc = np.asarray(W_fc, np.float32)
    b_fc = np.asarray(b_fc, np.float32)
    out = np.empty((B, C), np.float32)
    for c in range(N_CORES):
        mean_c = unpack_msum(res.results[c]["msum"], t_steps)  # [32, 512]
        out[c * BL:(c + 1) * BL] = mean_c @ W_fc.T + b_fc
    return out
